# revision 7
# baseline (speedup 1.0000x reference)
"""MoE routing kernel for Trainium2: softmax over 256 experts + top-8 per token.

Full input: gating_output [131072, 256] f32. Output: (topk_weights f32,
topk_indices int32), both [131072, 8] — matching jax.lax.top_k semantics
(values descending, ties broken by lowest index first).

Strategy: shard tokens row-wise across 8 NeuronCores (16384 tokens each; the
computation is row-local so no communication). Per core, tokens are processed
in chunks of T*128 tokens laid out as [128 partitions x T subtiles x 256
experts] with partition-contiguous DMA rows (token = chunk_base + p*T + t).
A short-prologue chunk schedule lets the compute engines start early.

Engine split per chunk (balancing DVE and ACT, the two busiest engines):
  DVE : T x InstMax (top-8 raw logits, descending), then T x InstMaxIndex
        (indices; duplicates get ascending distinct indices — matches
        jax.lax.top_k tie rules). Batched max-then-index ordering keeps
        dependent ops ~T instructions apart, hiding the DVE write-commit
        latency (~300 ns drain per dependent back-to-back pair). Also one
        reduce_sum for the last ACT_SPLIT subtiles' softmax denominators,
        and the reciprocal-free normalization inputs.
  ACT : per-subtile Exp with accum_out (softmax denominator via the ACT
        accumulator) for the first T-ACT_SPLIT subtiles; one plain Exp over
        the last ACT_SPLIT subtiles (cheaper per element, no accumulator
        drain) whose sums come from the DVE reduce; Exp on the [128, T*8]
        top-8 logits. Softmax max-subtraction is skipped: |x| <= ~5.5 so
        exp stays in f32 range, and softmax is shift-invariant.
  Pool: weights = exp(top8) * (1/sums) broadcast multiply.

Top-8 selection runs on raw logits (softmax is monotone, so same selection),
which avoids f32 ties introduced by exp rounding.
"""

import numpy as np

TOKENS = 131072
EXPERTS = 256
K = 8
N_CORES = 8
TOK_PER_CORE = TOKENS // N_CORES  # 16384
P = 128

# Subtile counts per chunk: short prologue so the first DMA lands fast and
# compute engines spin up early; steady-state 8-subtile (1 MiB) chunks.
CHUNKS = [2, 6] + [8] * 15
assert sum(CHUNKS) * P == TOK_PER_CORE

# Per chunk, the last ACT_SPLIT subtiles compute their softmax denominator
# via one plain ACT exp + a DVE reduce instead of per-subtile ACT exp+accum.
# Balances ACT (~68us) against DVE (~72us) per core.
ACT_SPLIT = 2

_PROGRAM_CACHE = {}


def _build_program():
    import concourse.tile as tile
    from concourse import bacc, mybir

    f32 = mybir.dt.float32
    u32 = mybir.dt.uint32
    Exp = mybir.ActivationFunctionType.Exp

    nc = bacc.Bacc("TRN2", debug=False, num_devices=N_CORES)

    g_dram = nc.dram_tensor(
        "gating", [TOK_PER_CORE, EXPERTS], f32, kind="ExternalInput"
    ).ap()
    w_dram = nc.dram_tensor(
        "weights", [TOK_PER_CORE, K], f32, kind="ExternalOutput"
    ).ap()
    i_dram = nc.dram_tensor(
        "indices", [TOK_PER_CORE, K], u32, kind="ExternalOutput"
    ).ap()

    with tile.TileContext(nc) as tc:
        with (
            tc.tile_pool(name="gin", bufs=4) as gin_pool,
            tc.tile_pool(name="expbuf", bufs=2) as exp_pool,
            tc.tile_pool(name="outs", bufs=3) as out_pool,
        ):
            base = 0
            for ci, T in enumerate(CHUNKS):
                rows = P * T
                nsplit = min(ACT_SPLIT, max(T - 1, 0))
                nacc = T - nsplit  # subtiles using ACT exp+accum
                # token = base + p*T + t: partition-contiguous T*1KiB rows
                g_c = g_dram[base : base + rows, :].rearrange(
                    "(p t) e -> p (t e)", p=P, t=T
                )
                w_c = w_dram[base : base + rows, :].rearrange(
                    "(p t) k -> p (t k)", p=P, t=T
                )
                i_c = i_dram[base : base + rows, :].rearrange(
                    "(p t) k -> p (t k)", p=P, t=T
                )
                base += rows

                gt = gin_pool.tile([P, T * EXPERTS], f32, name=f"gt{ci}", tag="gt")
                nc.sync.dma_start(out=gt, in_=g_c)
                gt3 = gt.rearrange("p (t e) -> p t e", t=T)

                vals = out_pool.tile([P, T, K], f32, name=f"vals{ci}", tag="vals")
                idxs = out_pool.tile([P, T, K], u32, name=f"idxs{ci}", tag="idxs")
                for t in range(T):
                    nc.vector.max(out=vals[:, t, :], in_=gt3[:, t, :])
                for t in range(T):
                    nc.vector.max_index(
                        out=idxs[:, t, :], in_max=vals[:, t, :], in_values=gt3[:, t, :]
                    )

                sums = out_pool.tile([P, T], f32, name=f"sums{ci}", tag="sums")
                # ACT accumulator route for the first nacc subtiles
                for t in range(nacc):
                    et = exp_pool.tile([P, EXPERTS], f32, name=f"et{ci}_{t}", tag="et")
                    nc.scalar.activation(
                        out=et,
                        in_=gt3[:, t, :],
                        func=Exp,
                        accum_out=sums[:, t : t + 1],
                    )
                # plain-exp + DVE-reduce route for the last nsplit subtiles
                if nsplit:
                    etw = exp_pool.tile(
                        [P, nsplit * EXPERTS], f32, name=f"etw{ci}", tag="etw"
                    )
                    nc.scalar.activation(
                        out=etw, in_=gt[:, nacc * EXPERTS :], func=Exp
                    )
                    nc.vector.reduce_sum(
                        out=sums[:, nacc:T],
                        in_=etw.rearrange("p (t e) -> p t e", t=nsplit),
                        axis=mybir.AxisListType.X,
                    )

                evals = out_pool.tile([P, T, K], f32, name=f"ev{ci}", tag="ev")
                nc.scalar.activation(out=evals, in_=vals, func=Exp)

                recips = out_pool.tile([P, T], f32, name=f"rec{ci}", tag="rec")
                nc.vector.reciprocal(recips, sums)

                wts = out_pool.tile([P, T, K], f32, name=f"wts{ci}", tag="wts")
                nc.gpsimd.tensor_tensor(
                    out=wts,
                    in0=evals,
                    in1=recips.rearrange("p (t one) -> p t one", one=1).to_broadcast(
                        [P, T, K]
                    ),
                    op=mybir.AluOpType.mult,
                )

                nc.sync.dma_start(out=w_c, in_=wts.rearrange("p t k -> p (t k)"))
                nc.sync.dma_start(out=i_c, in_=idxs.rearrange("p t k -> p (t k)"))

    nc.compile()
    return nc


def kernel(**inputs) -> tuple:
    from concourse.bass_utils import run_bass_kernel_spmd

    gating = np.ascontiguousarray(np.asarray(inputs["gating_output"], dtype=np.float32))
    topk = int(np.asarray(inputs.get("topk", K)))
    assert topk == K, f"kernel hardcodes top-{K}, got topk={topk}"
    assert gating.shape == (TOKENS, EXPERTS), gating.shape

    if "nc" not in _PROGRAM_CACHE:
        _PROGRAM_CACHE["nc"] = _build_program()
    nc = _PROGRAM_CACHE["nc"]

    shards = gating.reshape(N_CORES, TOK_PER_CORE, EXPERTS)
    in_maps = [{"gating": shards[c]} for c in range(N_CORES)]
    res = run_bass_kernel_spmd(nc, in_maps, core_ids=list(range(N_CORES)))
    _PROGRAM_CACHE["last_results"] = res

    weights = np.concatenate([r["weights"] for r in res.results], axis=0)
    indices = np.concatenate([r["indices"] for r in res.results], axis=0)
    return weights.astype(np.float32, copy=False), indices.astype(np.int32, copy=False)


# revision 8
# speedup vs baseline: 1.0467x; 1.0467x over previous
"""MoE routing kernel for Trainium2: softmax over 256 experts + top-8 per token.

Full input: gating_output [131072, 256] f32. Output: (topk_weights f32,
topk_indices int32), both [131072, 8] — matching jax.lax.top_k semantics
(values descending, ties broken by lowest index first).

Strategy: shard tokens row-wise across 8 NeuronCores (16384 tokens each; the
computation is row-local so no communication). Per core, tokens are processed
in chunks of T*128 tokens laid out as [128 partitions x T subtiles x 256
experts] with partition-contiguous DMA rows (token = chunk_base + p*T + t).
A short-prologue chunk schedule lets the compute engines start early.

Engine split per chunk (balancing DVE and ACT, the two busiest engines):
  DVE : T x InstMax (top-8 raw logits, descending), then T x InstMaxIndex
        (indices; duplicates get ascending distinct indices — matches
        jax.lax.top_k tie rules). Batched max-then-index ordering keeps
        dependent ops ~T instructions apart, hiding DVE write-commit
        latency; the top-8 outputs are padded to a 128-byte stride per
        subtile so consecutive ops don't collide on an SBUF write sector.
        Also a reduce_sum for the last ACT_SPLIT subtiles' denominators
        (after Pool pre-halves them) and the reciprocal.
  ACT : per-subtile Exp with accum_out (softmax denominator via the ACT
        accumulator) for the first T-ACT_SPLIT subtiles; one plain Exp over
        the last ACT_SPLIT subtiles (cheaper per element, no accumulator
        drain); Exp on the [128, T*8] top-8 logits. Softmax max-subtraction
        is skipped: |x| <= ~5.5 keeps exp in f32 range, and softmax is
        shift-invariant.
  Pool: pre-halves the plain-exp'd subtiles (tensor_tensor add) so the DVE
        reduce scans half the elements, and the final weights multiply
        exp(top8) * (1/sums).

Top-8 selection runs on raw logits (softmax is monotone, so same selection),
which avoids f32 ties introduced by exp rounding.
"""

import numpy as np

TOKENS = 131072
EXPERTS = 256
K = 8
KPAD = 32  # 128-byte output sector per subtile (f32/u32)
N_CORES = 8
TOK_PER_CORE = TOKENS // N_CORES  # 16384
P = 128

# Subtile counts per chunk: short prologue so the first DMA lands fast and
# compute engines spin up early; steady-state 8-subtile (1 MiB) chunks.
CHUNKS = [1, 3, 4] + [8] * 15
assert sum(CHUNKS) * P == TOK_PER_CORE

# Per chunk, the last ACT_SPLIT subtiles compute their softmax denominator
# via one plain ACT exp + Pool halving + DVE reduce instead of per-subtile
# ACT exp+accum. Balances ACT against DVE per core.
ACT_SPLIT = 2

_PROGRAM_CACHE = {}


def _build_program():
    import concourse.tile as tile
    from concourse import bacc, mybir

    f32 = mybir.dt.float32
    u32 = mybir.dt.uint32
    Exp = mybir.ActivationFunctionType.Exp

    nc = bacc.Bacc("TRN2", debug=False, num_devices=N_CORES)

    g_dram = nc.dram_tensor(
        "gating", [TOK_PER_CORE, EXPERTS], f32, kind="ExternalInput"
    ).ap()
    w_dram = nc.dram_tensor(
        "weights", [TOK_PER_CORE, K], f32, kind="ExternalOutput"
    ).ap()
    i_dram = nc.dram_tensor(
        "indices", [TOK_PER_CORE, K], u32, kind="ExternalOutput"
    ).ap()

    with tile.TileContext(nc) as tc:
        with (
            tc.tile_pool(name="gin", bufs=5) as gin_pool,
            tc.tile_pool(name="expbuf", bufs=2) as exp_pool,
            tc.tile_pool(name="outs", bufs=3) as out_pool,
        ):
            base = 0
            for ci, T in enumerate(CHUNKS):
                rows = P * T
                nsplit = min(ACT_SPLIT, max(T - 1, 0))
                nacc = T - nsplit  # subtiles using ACT exp+accum
                # token = base + p*T + t: partition-contiguous T*1KiB rows
                g_c = g_dram[base : base + rows, :].rearrange(
                    "(p t) e -> p (t e)", p=P, t=T
                )
                w_c = w_dram[base : base + rows, :].rearrange(
                    "(p t) k -> p t k", p=P, t=T
                )
                i_c = i_dram[base : base + rows, :].rearrange(
                    "(p t) k -> p t k", p=P, t=T
                )
                base += rows

                gt = gin_pool.tile([P, T * EXPERTS], f32, name=f"gt{ci}", tag="gt")
                nc.sync.dma_start(out=gt, in_=g_c)
                gt3 = gt.rearrange("p (t e) -> p t e", t=T)

                vals = out_pool.tile([P, T, KPAD], f32, name=f"vals{ci}", tag="vals")
                idxs = out_pool.tile([P, T, KPAD], u32, name=f"idxs{ci}", tag="idxs")
                for t in range(T):
                    nc.vector.max(out=vals[:, t, :K], in_=gt3[:, t, :])
                for t in range(T):
                    nc.vector.max_index(
                        out=idxs[:, t, :K],
                        in_max=vals[:, t, :K],
                        in_values=gt3[:, t, :],
                    )

                sums = out_pool.tile([P, T], f32, name=f"sums{ci}", tag="sums")
                # ACT accumulator route for the first nacc subtiles
                for t in range(nacc):
                    et = exp_pool.tile([P, EXPERTS], f32, name=f"et{ci}_{t}", tag="et")
                    nc.scalar.activation(
                        out=et,
                        in_=gt3[:, t, :],
                        func=Exp,
                        accum_out=sums[:, t : t + 1],
                    )
                # plain-exp + Pool halving + DVE reduce for the last nsplit
                if nsplit:
                    etw = exp_pool.tile(
                        [P, nsplit * EXPERTS], f32, name=f"etw{ci}", tag="etw"
                    )
                    nc.scalar.activation(
                        out=etw, in_=gt[:, nacc * EXPERTS :], func=Exp
                    )
                    etw3 = etw.rearrange("p (t e) -> p t e", t=nsplit)
                    half = EXPERTS // 2
                    nc.gpsimd.tensor_tensor(
                        out=etw3[:, :, :half],
                        in0=etw3[:, :, :half],
                        in1=etw3[:, :, half:],
                        op=mybir.AluOpType.add,
                    )
                    nc.vector.reduce_sum(
                        out=sums[:, nacc:T],
                        in_=etw3[:, :, :half],
                        axis=mybir.AxisListType.X,
                    )

                evals = out_pool.tile([P, T, K], f32, name=f"ev{ci}", tag="ev")
                nc.scalar.activation(out=evals, in_=vals[:, :, :K], func=Exp)

                recips = out_pool.tile([P, T], f32, name=f"rec{ci}", tag="rec")
                nc.vector.reciprocal(recips, sums)

                wts = out_pool.tile([P, T, K], f32, name=f"wts{ci}", tag="wts")
                nc.gpsimd.tensor_tensor(
                    out=wts,
                    in0=evals,
                    in1=recips.rearrange("p (t one) -> p t one", one=1).to_broadcast(
                        [P, T, K]
                    ),
                    op=mybir.AluOpType.mult,
                )

                nc.sync.dma_start(out=w_c, in_=wts)
                nc.sync.dma_start(out=i_c, in_=idxs[:, :, :K])

    nc.compile()
    return nc


def kernel(**inputs) -> tuple:
    from concourse.bass_utils import run_bass_kernel_spmd

    gating = np.ascontiguousarray(np.asarray(inputs["gating_output"], dtype=np.float32))
    topk = int(np.asarray(inputs.get("topk", K)))
    assert topk == K, f"kernel hardcodes top-{K}, got topk={topk}"
    assert gating.shape == (TOKENS, EXPERTS), gating.shape

    if "nc" not in _PROGRAM_CACHE:
        _PROGRAM_CACHE["nc"] = _build_program()
    nc = _PROGRAM_CACHE["nc"]

    shards = gating.reshape(N_CORES, TOK_PER_CORE, EXPERTS)
    in_maps = [{"gating": shards[c]} for c in range(N_CORES)]
    res = run_bass_kernel_spmd(nc, in_maps, core_ids=list(range(N_CORES)))
    _PROGRAM_CACHE["last_results"] = res

    weights = np.concatenate([r["weights"] for r in res.results], axis=0)
    indices = np.concatenate([r["indices"] for r in res.results], axis=0)
    return weights.astype(np.float32, copy=False), indices.astype(np.int32, copy=False)


# revision 9
# speedup vs baseline: 1.0624x; 1.0149x over previous
"""MoE routing kernel for Trainium2: softmax over 256 experts + top-8 per token.

Full input: gating_output [131072, 256] f32. Output: (topk_weights f32,
topk_indices int32), both [131072, 8] — matching jax.lax.top_k semantics
(values descending, ties broken by lowest index first).

Strategy: shard tokens row-wise across 8 NeuronCores (16384 tokens each; the
computation is row-local so no communication). Per core, tokens are processed
in chunks of T*128 tokens laid out as [128 partitions x T subtiles x 256
experts] with partition-contiguous DMA rows (token = chunk_base + p*T + t).
A short-prologue chunk schedule lets the compute engines start early.

Engine split per chunk (balancing DVE and ACT, the two busiest engines):
  DVE : T x InstMax (top-8 raw logits, descending), then T x InstMaxIndex
        (indices; duplicates get ascending distinct indices — matches
        jax.lax.top_k tie rules). Batched max-then-index ordering keeps
        dependent ops ~T instructions apart, hiding DVE write-commit
        latency; the top-8 outputs are padded to a 128-byte stride per
        subtile so consecutive ops don't collide on an SBUF write sector.
        Also a reduce_sum for the last ACT_SPLIT subtiles' denominators
        (after Pool pre-halves them) and the reciprocal.
  ACT : per-subtile Exp with accum_out (softmax denominator via the ACT
        accumulator) for the first T-ACT_SPLIT subtiles; one plain Exp over
        the last ACT_SPLIT subtiles (cheaper per element, no accumulator
        drain); Exp on the [128, T*8] top-8 logits. Softmax max-subtraction
        is skipped: |x| <= ~5.5 keeps exp in f32 range, and softmax is
        shift-invariant.
  Pool: pre-halves the plain-exp'd subtiles (tensor_tensor add) so the DVE
        reduce scans half the elements, and the final weights multiply
        exp(top8) * (1/sums).

Top-8 selection runs on raw logits (softmax is monotone, so same selection),
which avoids f32 ties introduced by exp rounding.
"""

import numpy as np

TOKENS = 131072
EXPERTS = 256
K = 8
KPAD = 32  # 128-byte output sector per subtile (f32/u32)
N_CORES = 8
TOK_PER_CORE = TOKENS // N_CORES  # 16384
P = 128

# Subtile counts per chunk: short prologue so the first DMA lands fast and
# compute engines spin up early; steady-state 8-subtile (1 MiB) chunks.
CHUNKS = [1, 3, 4] + [8] * 15
assert sum(CHUNKS) * P == TOK_PER_CORE

# Per chunk, the last ACT_SPLIT subtiles compute their softmax denominator
# via one plain ACT exp + Pool halving + DVE reduce instead of per-subtile
# ACT exp+accum. 0 keeps the DVE (the bottleneck engine: ~720ns of
# max8+find_index8 per subtile is intrinsic) free of any sum work; ACT's
# ~77us stays below the ~94us DVE floor.
ACT_SPLIT = 0

_PROGRAM_CACHE = {}


def _build_program():
    import concourse.tile as tile
    from concourse import bacc, mybir

    f32 = mybir.dt.float32
    u32 = mybir.dt.uint32
    Exp = mybir.ActivationFunctionType.Exp

    nc = bacc.Bacc("TRN2", debug=False, num_devices=N_CORES)

    g_dram = nc.dram_tensor(
        "gating", [TOK_PER_CORE, EXPERTS], f32, kind="ExternalInput"
    ).ap()
    w_dram = nc.dram_tensor(
        "weights", [TOK_PER_CORE, K], f32, kind="ExternalOutput"
    ).ap()
    i_dram = nc.dram_tensor(
        "indices", [TOK_PER_CORE, K], u32, kind="ExternalOutput"
    ).ap()

    with tile.TileContext(nc) as tc:
        with (
            tc.tile_pool(name="gin", bufs=5) as gin_pool,
            tc.tile_pool(name="expbuf", bufs=2) as exp_pool,
            tc.tile_pool(name="outs", bufs=3) as out_pool,
        ):
            base = 0
            for ci, T in enumerate(CHUNKS):
                rows = P * T
                nsplit = min(ACT_SPLIT, max(T - 1, 0))
                nacc = T - nsplit  # subtiles using ACT exp+accum
                # token = base + p*T + t: partition-contiguous T*1KiB rows
                g_c = g_dram[base : base + rows, :].rearrange(
                    "(p t) e -> p (t e)", p=P, t=T
                )
                w_c = w_dram[base : base + rows, :].rearrange(
                    "(p t) k -> p t k", p=P, t=T
                )
                i_c = i_dram[base : base + rows, :].rearrange(
                    "(p t) k -> p t k", p=P, t=T
                )
                base += rows

                gt = gin_pool.tile([P, T * EXPERTS], f32, name=f"gt{ci}", tag="gt")
                nc.sync.dma_start(out=gt, in_=g_c)
                gt3 = gt.rearrange("p (t e) -> p t e", t=T)

                vals = out_pool.tile([P, T, KPAD], f32, name=f"vals{ci}", tag="vals")
                idxs = out_pool.tile([P, T, KPAD], u32, name=f"idxs{ci}", tag="idxs")
                for t in range(T):
                    nc.vector.max(out=vals[:, t, :K], in_=gt3[:, t, :])
                for t in range(T):
                    nc.vector.max_index(
                        out=idxs[:, t, :K],
                        in_max=vals[:, t, :K],
                        in_values=gt3[:, t, :],
                    )

                sums = out_pool.tile([P, T], f32, name=f"sums{ci}", tag="sums")
                # ACT accumulator route for the first nacc subtiles
                for t in range(nacc):
                    et = exp_pool.tile([P, EXPERTS], f32, name=f"et{ci}_{t}", tag="et")
                    nc.scalar.activation(
                        out=et,
                        in_=gt3[:, t, :],
                        func=Exp,
                        accum_out=sums[:, t : t + 1],
                    )
                # plain-exp + Pool halving + DVE reduce for the last nsplit
                if nsplit:
                    etw = exp_pool.tile(
                        [P, nsplit * EXPERTS], f32, name=f"etw{ci}", tag="etw"
                    )
                    nc.scalar.activation(
                        out=etw, in_=gt[:, nacc * EXPERTS :], func=Exp
                    )
                    etw3 = etw.rearrange("p (t e) -> p t e", t=nsplit)
                    half = EXPERTS // 2
                    nc.gpsimd.tensor_tensor(
                        out=etw3[:, :, :half],
                        in0=etw3[:, :, :half],
                        in1=etw3[:, :, half:],
                        op=mybir.AluOpType.add,
                    )
                    nc.vector.reduce_sum(
                        out=sums[:, nacc:T],
                        in_=etw3[:, :, :half],
                        axis=mybir.AxisListType.X,
                    )

                evals = out_pool.tile([P, T, K], f32, name=f"ev{ci}", tag="ev")
                nc.scalar.activation(out=evals, in_=vals[:, :, :K], func=Exp)

                recips = out_pool.tile([P, T], f32, name=f"rec{ci}", tag="rec")
                nc.vector.reciprocal(recips, sums)

                wts = out_pool.tile([P, T, K], f32, name=f"wts{ci}", tag="wts")
                nc.gpsimd.tensor_tensor(
                    out=wts,
                    in0=evals,
                    in1=recips.rearrange("p (t one) -> p t one", one=1).to_broadcast(
                        [P, T, K]
                    ),
                    op=mybir.AluOpType.mult,
                )

                nc.sync.dma_start(out=w_c, in_=wts)
                nc.sync.dma_start(out=i_c, in_=idxs[:, :, :K])

    nc.compile()
    return nc


def kernel(**inputs) -> tuple:
    from concourse.bass_utils import run_bass_kernel_spmd

    gating = np.ascontiguousarray(np.asarray(inputs["gating_output"], dtype=np.float32))
    topk = int(np.asarray(inputs.get("topk", K)))
    assert topk == K, f"kernel hardcodes top-{K}, got topk={topk}"
    assert gating.shape == (TOKENS, EXPERTS), gating.shape

    if "nc" not in _PROGRAM_CACHE:
        _PROGRAM_CACHE["nc"] = _build_program()
    nc = _PROGRAM_CACHE["nc"]

    shards = gating.reshape(N_CORES, TOK_PER_CORE, EXPERTS)
    in_maps = [{"gating": shards[c]} for c in range(N_CORES)]
    res = run_bass_kernel_spmd(nc, in_maps, core_ids=list(range(N_CORES)))
    _PROGRAM_CACHE["last_results"] = res

    weights = np.concatenate([r["weights"] for r in res.results], axis=0)
    indices = np.concatenate([r["indices"] for r in res.results], axis=0)
    return weights.astype(np.float32, copy=False), indices.astype(np.int32, copy=False)


# revision 10
# speedup vs baseline: 1.1058x; 1.0409x over previous
"""MoE routing kernel for Trainium2: softmax over 256 experts + top-8 per token.

Full input: gating_output [131072, 256] f32. Output: (topk_weights f32,
topk_indices int32), both [131072, 8] — matching jax.lax.top_k semantics
(values descending, ties broken by lowest index first).

Strategy: shard tokens row-wise across 8 NeuronCores (16384 tokens each; the
computation is row-local so no communication). Per core, token = p*128 + tt
(partition-major): partition p owns 128 consecutive tokens, processed in
chunks of T subtiles (T consecutive token rows per partition, so each chunk's
input DMA is 128 descriptors of T KiB contiguous). A short-prologue chunk
schedule lets the compute engines start early.

Engine split per chunk:
  DVE : T x InstMax (top-8 raw logits, descending), then T x InstMaxIndex
        (indices; duplicates get ascending distinct indices — matches
        jax.lax.top_k tie rules), plus a tiny reciprocal. This is the
        bottleneck engine: ~721 ns per subtile is the ISA-model floor.
  ACT : per-subtile Exp with accum_out (softmax denominator via the ACT
        accumulator). Softmax max-subtraction is skipped: |x| <= ~5.5 keeps
        exp well inside f32 range, and softmax is shift-invariant. Also Exp
        on the [128, T*8] top-8 logits.
  Pool: final weights multiply exp(top8) * (1/sums).

Top-k results accumulate in persistent SBUF buffers and flush to DRAM in
quarter-core batches (4 KiB-contiguous runs per partition) so output DMA is
a few large-descriptor transfers instead of thousands of 256 B ones.

Top-8 selection runs on raw logits (softmax is monotone, so same selection),
which avoids f32 ties introduced by exp rounding.
"""

import numpy as np

TOKENS = 131072
EXPERTS = 256
K = 8
N_CORES = 8
TOK_PER_CORE = TOKENS // N_CORES  # 16384
P = 128
TT = TOK_PER_CORE // P  # 128 token rows per partition

# Subtile counts per chunk: short prologue so the first DMA lands fast and
# compute engines spin up early; steady-state 8-subtile (1 MiB) chunks.
CHUNKS = [1, 3, 4] + [8] * 15
assert sum(CHUNKS) == TT

# Flush the persistent output buffers after these many accumulated subtiles.
FLUSH_AT = (32, 64, 96, TT)

_PROGRAM_CACHE = {}


def _build_program():
    import concourse.tile as tile
    from concourse import bacc, mybir

    f32 = mybir.dt.float32
    u32 = mybir.dt.uint32
    Exp = mybir.ActivationFunctionType.Exp

    nc = bacc.Bacc("TRN2", debug=False, num_devices=N_CORES)

    g_dram = nc.dram_tensor(
        "gating", [TOK_PER_CORE, EXPERTS], f32, kind="ExternalInput"
    ).ap()
    w_dram = nc.dram_tensor(
        "weights", [TOK_PER_CORE, K], f32, kind="ExternalOutput"
    ).ap()
    i_dram = nc.dram_tensor(
        "indices", [TOK_PER_CORE, K], u32, kind="ExternalOutput"
    ).ap()

    # token = p*TT + tt: partition-major views
    g_v = g_dram.rearrange("(p tt) e -> p tt e", p=P)  # [128, 128, 256]
    w_v = w_dram.rearrange("(p tt) k -> p tt k", p=P)  # [128, 128, 8]
    i_v = i_dram.rearrange("(p tt) k -> p tt k", p=P)

    with tile.TileContext(nc) as tc:
        with (
            tc.tile_pool(name="gin", bufs=5) as gin_pool,
            tc.tile_pool(name="expbuf", bufs=2) as exp_pool,
            tc.tile_pool(name="outs", bufs=3) as out_pool,
            tc.tile_pool(name="persist", bufs=1) as persist_pool,
        ):
            # persistent per-core result buffers (8 KiB/partition total)
            wbuf = persist_pool.tile([P, TT, K], f32, name="wbuf")
            ibuf = persist_pool.tile([P, TT, K], u32, name="ibuf")

            ct = 0
            flushed = 0
            fi = 0
            for ci, T in enumerate(CHUNKS):
                gt = gin_pool.tile([P, T * EXPERTS], f32, name=f"gt{ci}", tag="gt")
                nc.sync.dma_start(out=gt, in_=g_v[:, ct : ct + T, :])
                gt3 = gt.rearrange("p (t e) -> p t e", t=T)

                vals = out_pool.tile([P, T, K], f32, name=f"vals{ci}", tag="vals")
                for t in range(T):
                    nc.vector.max(out=vals[:, t, :], in_=gt3[:, t, :])
                for t in range(T):
                    nc.vector.max_index(
                        out=ibuf[:, ct + t, :],
                        in_max=vals[:, t, :],
                        in_values=gt3[:, t, :],
                    )

                sums = out_pool.tile([P, T], f32, name=f"sums{ci}", tag="sums")
                for t in range(T):
                    et = exp_pool.tile([P, EXPERTS], f32, name=f"et{ci}_{t}", tag="et")
                    nc.scalar.activation(
                        out=et,
                        in_=gt3[:, t, :],
                        func=Exp,
                        accum_out=sums[:, t : t + 1],
                    )

                evals = out_pool.tile([P, T, K], f32, name=f"ev{ci}", tag="ev")
                nc.scalar.activation(out=evals, in_=vals, func=Exp)

                recips = out_pool.tile([P, T], f32, name=f"rec{ci}", tag="rec")
                nc.vector.reciprocal(recips, sums)

                nc.gpsimd.tensor_tensor(
                    out=wbuf[:, ct : ct + T, :],
                    in0=evals,
                    in1=recips.rearrange("p (t one) -> p t one", one=1).to_broadcast(
                        [P, T, K]
                    ),
                    op=mybir.AluOpType.mult,
                )

                ct += T
                if fi < len(FLUSH_AT) and ct >= FLUSH_AT[fi]:
                    nc.sync.dma_start(
                        out=w_v[:, flushed:ct, :], in_=wbuf[:, flushed:ct, :]
                    )
                    nc.sync.dma_start(
                        out=i_v[:, flushed:ct, :], in_=ibuf[:, flushed:ct, :]
                    )
                    flushed = ct
                    fi += 1

    nc.compile()
    return nc


def kernel(**inputs) -> tuple:
    from concourse.bass_utils import run_bass_kernel_spmd

    gating = np.ascontiguousarray(np.asarray(inputs["gating_output"], dtype=np.float32))
    topk = int(np.asarray(inputs.get("topk", K)))
    assert topk == K, f"kernel hardcodes top-{K}, got topk={topk}"
    assert gating.shape == (TOKENS, EXPERTS), gating.shape

    if "nc" not in _PROGRAM_CACHE:
        _PROGRAM_CACHE["nc"] = _build_program()
    nc = _PROGRAM_CACHE["nc"]

    shards = gating.reshape(N_CORES, TOK_PER_CORE, EXPERTS)
    in_maps = [{"gating": shards[c]} for c in range(N_CORES)]
    res = run_bass_kernel_spmd(nc, in_maps, core_ids=list(range(N_CORES)))
    _PROGRAM_CACHE["last_results"] = res

    weights = np.concatenate([r["weights"] for r in res.results], axis=0)
    indices = np.concatenate([r["indices"] for r in res.results], axis=0)
    return weights.astype(np.float32, copy=False), indices.astype(np.int32, copy=False)
